# revision 3
# baseline (speedup 1.0000x reference)
"""Embedding lookup (weight[indices]) on 8 TRN2 NeuronCores.

Strategy: replicate the 1M x 128 f32 table in each core's HBM, shard the
4096*200 = 819200 indices 8 ways (data parallel).  Each core loops over
tiles of 128*K indices: SWDGE indirect DMAs gather rows (512 B each,
one offset per partition per instruction — the only pattern the
INDIRECT1D ucode supports) from the HBM table into an SBUF tile
[128, K*128]; an HWDGE DMA stores the tile to the core's contiguous
output shard.  The gathers are round-robined across all 4 SWDGE queues:
descriptor generation (~1 us fixed per instruction) is the bottleneck
on a single queue, and the queues process in parallel on the Q7.
"""

import numpy as np

NUM_EMB = 1_000_000
D = 128
N_CORES = 8
P = 128

# tuning knobs
K = 50        # rows gathered per partition per tile -> tile = [128, K*128] f32
BUFS = 4      # SBUF tile double-buffering depth
N_QUEUES = 4  # SWDGE queues to spread indirect gathers across

_CACHE = {}

_QUEUE_NAMES = ["qPoolDynamic", "qPoolDynamic1", "qPoolDynamic2", "qPoolDynamic3"]


def _indirect_gather_q(gp, out, in_, offset_ap, queue_name):
    """nc.gpsimd.indirect_dma_start (gather direction), with queue choice.

    Mirrors bass.BassEngine.indirect_dma_start: one offset per partition,
    each gathering a run of len(out)//len(offsets) contiguous elements
    from `in_` starting at offset*row_len.  The stock method hardcodes
    queue="qPoolDynamic"; spreading across the 4 SWDGE queues is the
    whole point here.
    """
    import concourse.mybir as mybir

    assert isinstance(in_.offset, int) and in_.offset == 0
    out_ap = gp.lower_ap_dma(out, for_indirect_dma=True)
    in_ap = gp.lower_ap_dma(in_, for_indirect_dma=True)
    assert len(in_ap) == 1 and len(out_ap) == 1
    off = gp.lower_ap_dma(offset_ap)
    assert len(off) == 1
    in_ap.append(off[0])

    ap_shape = in_.shape
    coef = 1
    for i in range(1, len(ap_shape)):
        coef *= ap_shape[i]
    in_ap[0].dynamic_ap_info = mybir.DynamicAccessPatternInfo(
        c=0,
        actual_ap=out.ap,
        indirect_dim_max_index=ap_shape[0],
        offset_expr=[
            mybir.DynamicAccessPatternOffsetExpr(
                coef=coef,
                aff_expr=mybir.DynamicAccessPatternOffsetExprAffExpr(
                    kind="IndirectArgId", arg_id=1
                ),
            )
        ],
    )
    return gp.add_instruction(
        mybir.InstDMACopy(
            name=gp.bass.get_next_instruction_name(),
            queue=queue_name,
            mode="Copy",
            ins=in_ap,
            outs=out_ap,
            oob_is_err=True,
            cce_op=mybir.AluOpType.bypass,
        )
    )


def _build_bass(per_core: int, k: int, bufs: int, num_emb: int = NUM_EMB):
    import concourse.bacc as bacc
    import concourse.mybir as mybir
    import concourse.tile as tile

    key = (per_core, k, bufs, num_emb)
    if key in _CACHE:
        return _CACHE[key]

    nc = bacc.Bacc(
        "TRN2",
        target_bir_lowering=False,
        debug=False,
        num_devices=N_CORES,
        num_swdge_queues=N_QUEUES,
    )
    idx = nc.dram_tensor("idx", [per_core], mybir.dt.int32, kind="ExternalInput")
    weight = nc.dram_tensor(
        "weight", [num_emb, D], mybir.dt.float32, kind="ExternalInput"
    )
    out = nc.dram_tensor("out", [per_core, D], mybir.dt.float32, kind="ExternalOutput")

    n_per_part = per_core // P            # indices each partition handles
    assert per_core == n_per_part * P
    n_tiles = n_per_part // k
    assert n_per_part == n_tiles * k

    with tile.TileContext(nc) as tc:
        with (
            tc.tile_pool(name="idxp", bufs=1) as idxp,
            tc.tile_pool(name="data", bufs=bufs) as datap,
        ):
            idx_tile = idxp.tile([P, n_per_part], mybir.dt.int32)
            nc.sync.dma_start(idx_tile[:], idx[:].rearrange("(p n) -> p n", p=P))
            out_r = out[:].rearrange("(p n) d -> p (n d)", p=P)

            import concourse.bass as bass

            q = 0
            for t in range(n_tiles):
                dtile = datap.tile([P, k * D], mybir.dt.float32)
                for j in range(k):
                    n = t * k + j
                    _indirect_gather_q(
                        nc.gpsimd,
                        dtile[:, j * D : (j + 1) * D],
                        weight[:],
                        idx_tile[:, n : n + 1],
                        _QUEUE_NAMES[q % N_QUEUES],
                    )
                    q += 1
                nc.sync.dma_start(
                    out_r[:, t * k * D : (t + 1) * k * D], dtile[:]
                )
    nc.compile()
    _CACHE[key] = nc
    return nc


def run_sharded(indices: np.ndarray, weight: np.ndarray, trace: bool = False):
    """Shard indices across 8 cores, run the Bass kernel, return
    (full_output, BassKernelResults)."""
    from concourse.bass_utils import run_bass_kernel_spmd

    idx_flat = np.ascontiguousarray(indices.reshape(-1).astype(np.int32))
    w = np.ascontiguousarray(weight, dtype=np.float32)
    n_idx = idx_flat.shape[0]
    per_core = n_idx // N_CORES
    assert n_idx == per_core * N_CORES

    nc = _build_bass(per_core, K, BUFS)
    in_maps = [
        {"idx": idx_flat[c * per_core : (c + 1) * per_core], "weight": w}
        for c in range(N_CORES)
    ]
    res = run_bass_kernel_spmd(
        nc, in_maps, core_ids=list(range(N_CORES)), trace=trace
    )
    # per-core output rows are ordered [p * n_per_part + n] -> global order
    # within the shard matches the input order (we sharded contiguously).
    full = np.concatenate([r["out"] for r in res.results], axis=0)
    return full.reshape(indices.shape + (D,)), res


def kernel(indices: np.ndarray, weight: np.ndarray) -> np.ndarray:
    full, _ = run_sharded(indices, weight, trace=False)
    return full
